# revision 1
# baseline (speedup 1.0000x reference)
"""LoLa message-passing kernel for 8 Trainium2 NeuronCores.

Math (algebraically identical to the reference):
  ch0 masses      = f3^2 - f0^2 - f1^2 - f2^2
  ch1 ptsq        = f1^2 + f2^2
  ch2 w_ener@f0, ch4 w_pid@f3, ch5 w_extra0@f4, ch6 w_extra1@f5
  ch3 weighted_d  = masses * rowsum(w_dist) + w_dist @ masses
                    + 2*(f0*(w_dist@f0) + f1*(w_dist@f1)
                         + f2*(w_dist@f2) - f3*(w_dist@f3))

Sharding: model-parallel over particles N (64 output rows per core); combvec
replicated (full contraction operand), weights sliced 1/8 per core.

Device-side design notes:
 - Single-term bf16 everywhere (fp32 PSUM accumulate): rel err ~3.6e-3,
   well under the 2e-2 gate, at half the HBM bytes and a third of the
   matmul count of an fp32-faithful hi/lo split.
 - The input stream is byte-rate-bound (~160 GB/s aggregate with all 8
   cores pulling, regardless of queue count or descriptor size). The two
   HWDGE queues are byte-balanced (589/588 KB) and ordered so weights
   land first (every matmul needs them) and ft chunk 2 lands last.
 - Stationary pairs pack two 64-row weight slices side by side; w_dist is
   stored once (C2 reuses pair 0's stationary):
     MM-A: [w_dist | w_ener]  @ [f0|f1|f2|f3]        (512 cols -> psA)
     MM-B: [w_pid  | w_extra0]@ [f3|f4]              (256 cols -> psB)
     MM-C1: w_extra1 (64-col stationary) @ f5        (128 cols -> psC1 lo)
     MM-C2: [w_dist | w_ener] @ [masses|1]           (129 cols -> psC2)
 - masses per 128-row chunk: square + 3 chained subtracts in bf16, written
   straight into the mt tile. Chunks 0+1 as one strided 2-chunk batch on
   vector; chunks 2/3 squares on the scalar ACT engine, combines on
   vector/gpsimd (tensor_reduce measured slower than tt chains).
 - fr (this core's 64 rows of f0..f3) ships bf16 with f3 negated on the
   host, so this core's masses/ptsq squares and the quad combine are
   sign-uniform adds.
 - tc.tile_wait_until stamps force the Tile scheduler's per-engine order
   to match real DMA arrival times (its cost model assumes instant DMAs
   and otherwise schedules the quad chain ahead of the masses combines
   that gate psC2).
"""

import sys

if "/opt/trn_rl_repo" not in sys.path:
    sys.path.insert(0, "/opt/trn_rl_repo")

import numpy as np
import ml_dtypes

import concourse.bass as bass
import concourse.mybir as mybir
import concourse.tile as tile
from concourse import bacc
from concourse.bass_utils import run_bass_kernel_spmd

B, N, F = 128, 512, 6
NCORES = 8
NS = N // NCORES  # 64 output rows per core
KC = N // 128  # 4 contraction chunks of 128
FW = 768  # ft cols per chunk: 6 features x 128 batch
MW = 132  # mt cols per chunk: 128 masses | 1 one | 3 pad
PW = 320  # wt cols per chunk: [w_dist|w_ener] 128, [w_pid|w_x0] 128, w_x1 64
DT = mybir.dt.float32
BF = mybir.dt.bfloat16
ALU = mybir.AluOpType
ACTF = mybir.ActivationFunctionType


def _emit(tc, nc, ft_d, wt_d, fr_d, out_d):
    with (
        tc.tile_pool(name="sbuf", bufs=1) as sb,
        tc.tile_pool(name="scratch", bufs=4) as scr,
        tc.tile_pool(name="psum", bufs=1, space="PSUM") as ps,
    ):
        # --- persistent SBUF tiles ---
        ft = sb.tile([128, KC * FW], BF)  # features [c*768 + k*128 + b]
        mt = sb.tile([128, KC * MW], BF)  # masses [c*132 + b], ones at c*132+128
        wt = sb.tile([128, KC * PW], BF)  # weight pairs [c*320 + ...]
        fr = sb.tile([64, 4 * B], BF)  # this core's n-rows of f0..f3 (f3 negated)
        frsq = sb.tile([64, 4 * B], DT)
        quad = sb.tile([64, 4 * B], BF)  # bf16: 2x DVE rate; feeds 2*qsum only
        qs = sb.tile([64, 2 * B], BF)
        tmp3 = sb.tile([64, B], DT)
        tmp3f = sb.tile([64, B], DT)
        olo = sb.tile([64, 5 * B], DT)  # out staging ch 0,1,3,4,6 (partitions 0:64)
        ohi = sb.tile([128, 2 * B], DT)  # out staging ch 2,5 (partitions 64:128)

        # --- PSUM tiles ---
        psA = ps.tile([128, 512], DT)  # [dist|ener] @ [f0|f1|f2|f3]
        psB = ps.tile([128, 256], DT)  # [pid|x0]   @ [f3|f4]
        psC1 = ps.tile([64, B], DT)  # x1 @ f5 (64-col stationary -> lo partitions)
        psC2 = ps.tile([128, 132], DT)  # [dist|ener] @ [m|1] (dist/rowsum rows :64)

        # --- DMAs in: byte-balanced across the two HWDGE queues (each
        # sustains ~85-110 GB/s; together ~160). sync: ft chunks 0+1 then 3;
        # scalar: fr (feeds the early gpsimd chain), weights, ft chunk 2. ---
        nc.sync.dma_start(ft[:, 0: 2 * FW], ft_d[:, 0: 2 * FW])
        nc.scalar.dma_start(wt[:], wt_d[:])
        nc.scalar.dma_start(fr[:], fr_d[:])
        nc.sync.dma_start(ft[:, 3 * FW:], ft_d[:, 3 * FW:])
        nc.scalar.dma_start(ft[:, 2 * FW: 3 * FW], ft_d[:, 2 * FW: 3 * FW])

        # ones column at [c*MW + 128] (vector's first op; no DMA dep)
        mt4 = mt[:].rearrange("p (c x) -> p c x", c=KC, x=MW)
        nc.vector.memset(mt4[:, :, 128:129], 1.0)

        # --- PE warm-up: dep-free dummy matmuls give HAM sustained activity
        # from kernel start until the weights land (~12us), so the clock
        # ramps (1.2 -> 2.4 GHz) before the real matmul crunch. Also
        # tightens run-to-run clock-state variance. ---
        warm = sb.tile([128, 2 * B], BF)
        psW = ps.tile([128, 512], DT)
        nc.vector.memset(warm[:], 0.5)
        wmov = warm[:, None, :].to_broadcast([128, 4, 2 * B])
        for i in range(7):
            nc.tensor.matmul(
                psW[:], warm[:, 0:B], wmov[:, :, 0:B], start=i == 0, stop=i == 6
            )

        # --- masses: sq = f*f (bf16), m = sq3-sq2-sq1-sq0 chained, last
        # subtract lands in mt. Chunks 0+1 as one strided 2-chunk batch on
        # vector (halves per-op overhead); chunk 2 on vector, chunk 3 on
        # gpsimd (free after the early fr chain). ---
        sq01 = scr.tile([128, 1024], BF, name="sq01")
        t01 = scr.tile([128, 2 * B], BF, name="t01")
        ftv = ft[:, 0: 2 * FW].rearrange("p (cc x) -> p cc x", cc=2, x=FW)
        sqv = sq01[:].rearrange("p (cc x) -> p cc x", cc=2, x=512)
        tv = t01[:].rearrange("p (cc x) -> p cc x", cc=2, x=B)
        mtv = mt[:, 0: 2 * MW].rearrange("p (cc x) -> p cc x", cc=2, x=MW)
        nc.vector.tensor_tensor(
            out=sq01[:], in0=ftv[:, :, 0:512], in1=ftv[:, :, 0:512], op=ALU.mult
        )
        nc.vector.tensor_tensor(
            out=t01[:], in0=sqv[:, :, 3 * B: 4 * B], in1=sqv[:, :, 2 * B: 3 * B],
            op=ALU.subtract,
        )
        nc.vector.tensor_tensor(
            out=t01[:], in0=t01[:], in1=sqv[:, :, B: 2 * B], op=ALU.subtract
        )
        nc.vector.tensor_tensor(
            out=mtv[:, :, 0:B], in0=tv, in1=sqv[:, :, 0:B], op=ALU.subtract
        )
        # chunks 2+3: squares on scalar ACT (otherwise idle mid-kernel);
        # combines on vector (c2) / gpsimd (c3). high_priority hints the
        # scheduler to place the combines ahead of the quad chain, which it
        # otherwise reorders in front of them (its cost model does not see
        # real DMA arrival times).
        sq2 = scr.tile([128, 4 * B], BF, name="sq2")
        t2 = scr.tile([128, B], BF, name="t2")
        sq3 = scr.tile([128, 4 * B], BF, name="sq3")
        t3 = scr.tile([128, B], BF, name="t3")
        # scalar queue order matches real arrival: ft3 (~11.5us) -> fr
        # (~12us) -> ft2 (last, ~14.4us).
        nc.scalar.activation(sq3[:], ft[:, 3 * FW: 3 * FW + 512], ACTF.Square)
        nc.scalar.activation(frsq[:], fr[:], ACTF.Square)
        nc.scalar.activation(sq2[:], ft[:, 2 * FW: 2 * FW + 512], ACTF.Square)
        # c3 combines on vector (ft3 arrives early on the sync queue; they
        # fit vector's idle window before sq2 lands). The LAST-arriving
        # chunk (ft2) gets its combines on the otherwise-idle gpsimd, in
        # parallel with vector's quad chain.
        with tc.tile_wait_until(1):
            nc.vector.tensor_tensor(
                out=t3[:], in0=sq3[:, 3 * B: 4 * B], in1=sq3[:, 2 * B: 3 * B],
                op=ALU.subtract,
            )
            nc.vector.tensor_tensor(
                out=t3[:], in0=t3[:], in1=sq3[:, B: 2 * B], op=ALU.subtract
            )
            nc.vector.tensor_tensor(
                out=mt[:, 3 * MW: 3 * MW + B], in0=t3[:], in1=sq3[:, 0:B],
                op=ALU.subtract,
            )
        # this core's ch0/ch1 combines fill vector's idle window between
        # m_c01 and the sq2-gated c2 combines (on gpsimd they serialized
        # behind the c3 combines and delayed ch0 past the stt chain).
        with tc.tile_wait_until(2):
            nc.vector.tensor_tensor(
                out=olo[:, B: 2 * B], in0=frsq[:, B: 2 * B],
                in1=frsq[:, 2 * B: 3 * B], op=ALU.add,
            )
            nc.vector.tensor_tensor(
                out=tmp3[:], in0=frsq[:, 3 * B: 4 * B], in1=frsq[:, 0:B],
                op=ALU.subtract,
            )
            nc.vector.tensor_tensor(
                out=olo[:, 0:B], in0=tmp3[:], in1=olo[:, B: 2 * B], op=ALU.subtract
            )
        with tc.tile_wait_until(3):
            nc.gpsimd.tensor_tensor(
                out=t2[:], in0=sq2[:, 3 * B: 4 * B], in1=sq2[:, 2 * B: 3 * B],
                op=ALU.subtract,
            )
            nc.gpsimd.tensor_tensor(
                out=t2[:], in0=t2[:], in1=sq2[:, B: 2 * B], op=ALU.subtract
            )
            nc.gpsimd.tensor_tensor(
                out=mt[:, 2 * MW: 2 * MW + B], in0=t2[:], in1=sq2[:, 0:B],
                op=ALU.subtract,
            )

        # --- matmuls: A/B/C1 for all chunks first (DMA-gated only), then the
        # masses-dependent C2s (reusing pair 0's stationary). ---
        def mmABC(c):
            fb = c * FW
            wb = c * PW
            nc.tensor.matmul(
                psA[:], wt[:, wb: wb + 128], ft[:, fb: fb + 512],
                start=c == 0, stop=c == 2,
            )
            nc.tensor.matmul(
                psB[:], wt[:, wb + 128: wb + 256], ft[:, fb + 384: fb + 640],
                start=c == 0, stop=c == 2,
            )
            nc.tensor.matmul(
                psC1[:], wt[:, wb + 256: wb + 320], ft[:, fb + 640: fb + 768],
                start=c == 0, stop=c == 2,
            )

        def mmC2(c, start=False, stop=False):
            nc.tensor.matmul(
                psC2[:, 0:129], wt[:, c * PW: c * PW + 128],
                mt[:, c * MW: c * MW + 129],
                start=start, stop=stop,
            )

        # PE order follows real DMA arrival (ft chunk 3 lands before ft
        # chunk 2, which is last); C2 c0/c1 squeeze in between. Stop flags
        # sit on chunk 2's matmuls (last executed of each group).
        mmABC(0)
        mmABC(1)
        mmABC(3)
        mmC2(0, start=True)
        mmC2(1)
        mmABC(2)
        with tc.tile_wait_until(1):
            mmC2(3)
        with tc.tile_wait_until(2):
            mmC2(2, stop=True)

        # --- quad chain on vector: fr * psA rows is all-additive thanks to
        # the host-side f3 negation; tt adds + two fused stt ops finish ch3
        # (tensor_reduce measured slower than chained tt). ---
        with tc.tile_wait_until(4):
            nc.vector.tensor_tensor(
                out=quad[:], in0=fr[:], in1=psA[0:64, :], op=ALU.mult
            )
            nc.vector.tensor_tensor(
                out=qs[:, 0: 2 * B], in0=quad[:, 0: 2 * B],
                in1=quad[:, 2 * B: 4 * B], op=ALU.add,
            )
            nc.vector.tensor_tensor(
                out=qs[:, 0:B], in0=qs[:, 0:B], in1=qs[:, B: 2 * B], op=ALU.add
            )
        with tc.tile_wait_until(5):
            nc.vector.scalar_tensor_tensor(
                out=tmp3f[:],
                in0=olo[:, 0:B],
                scalar=psC2[0:64, 128:129],
                in1=psC2[0:64, 0:B],
                op0=ALU.mult,
                op1=ALU.add,
            )
            nc.vector.scalar_tensor_tensor(
                out=olo[:, 2 * B: 3 * B],
                in0=qs[:, 0:B],
                scalar=2.0,
                in1=tmp3f[:],
                op0=ALU.mult,
                op1=ALU.add,
            )
        # ch4 = w_pid@f3 and ch6 = w_x1@f5 (low partitions); ch2/ch5 (high)
        nc.scalar.copy(olo[:, 3 * B: 4 * B], psB[0:64, 0:B])
        nc.scalar.copy(olo[:, 4 * B: 5 * B], psC1[:, 0:B])
        nc.scalar.copy(ohi[64:128, 0:B], psA[64:128, 0:B])  # ch2 ener
        nc.scalar.copy(ohi[64:128, B: 2 * B], psB[64:128, B: 2 * B])  # ch5 x0

        # --- DMAs out, staggered by readiness on the idle sync queue so the
        # FINAL transfer (whose completion receipt gates the postamble) is
        # just ch3's 32KB: ch0/ch1 go right after the fr combines (~14.5us,
        # input stream already drained), ch4/ch6 after the PSUM copies,
        # ch3 last after stt2. ohi (ch2/ch5) rides the scalar queue. ---
        nc.scalar.dma_start(out_d[:, 5 * B: 7 * B], ohi[64:128, :])
        with tc.tile_wait_until(3):
            nc.sync.dma_start(out_d[:, 0: 2 * B], olo[:, 0: 2 * B])
        with tc.tile_wait_until(5):
            nc.sync.dma_start(out_d[:, 3 * B: 5 * B], olo[:, 3 * B: 5 * B])
        with tc.tile_wait_until(6):
            nc.sync.dma_start(out_d[:, 2 * B: 3 * B], olo[:, 2 * B: 3 * B])


_NC_CACHE = {}


def _get_nc():
    if "nc" not in _NC_CACHE:
        nc = bacc.Bacc(
            "TRN2", target_bir_lowering=False, debug=False, num_devices=NCORES
        )
        ft_d = nc.dram_tensor("ft", [128, KC * FW], BF, kind="ExternalInput")
        wt_d = nc.dram_tensor("wt", [128, KC * PW], BF, kind="ExternalInput")
        fr_d = nc.dram_tensor("fr", [64, 4 * B], BF, kind="ExternalInput")
        out_d = nc.dram_tensor("out", [64, 7 * B], DT, kind="ExternalOutput")
        with tile.TileContext(nc) as tc:
            _emit(tc, nc, ft_d.ap(), wt_d.ap(), fr_d.ap(), out_d.ap())
        nc.compile()
        _NC_CACHE["nc"] = nc
    return _NC_CACHE["nc"]


def make_in_maps(combvec, w_dist, w_ener, w_pid, w_extra0, w_extra1):
    ft_t = np.ascontiguousarray(
        np.transpose(np.asarray(combvec, np.float32), (2, 1, 0))
    )  # (6, 512, 128) [k, m, b]
    # ft layout: [p, c*768 + k*128 + b] = ft_t[k, c*128+p, b]
    ftfull = np.ascontiguousarray(
        ft_t.reshape(F, KC, 128, B).transpose(2, 1, 0, 3)
    ).reshape(128, KC * FW)
    ft_np = ftfull.astype(ml_dtypes.bfloat16)

    weights = {
        "w_dist": np.asarray(w_dist, np.float32),
        "w_pid": np.asarray(w_pid, np.float32),
        "w_ener": np.asarray(w_ener, np.float32),
        "w_extra0": np.asarray(w_extra0, np.float32),
        "w_extra1": np.asarray(w_extra1, np.float32),
    }
    in_maps = []
    for core in range(NCORES):
        sl = slice(NS * core, NS * (core + 1))
        # per weight: slice (64, 512) -> transposed chunks (c, p, n) = (4,128,64)
        wch = {
            k: w[sl].T.reshape(KC, 128, NS) for k, w in weights.items()
        }
        # wt layout per chunk: [w_dist|w_ener] (128), [w_pid|w_x0] (128), w_x1 (64)
        wt_ = np.concatenate(
            [
                np.concatenate(
                    [wch["w_dist"], wch["w_ener"], wch["w_pid"],
                     wch["w_extra0"], wch["w_extra1"]], axis=2
                )  # (c, p, 320)
            ],
            axis=2,
        ).transpose(1, 0, 2).reshape(128, KC * PW)
        wt_np = np.ascontiguousarray(wt_).astype(ml_dtypes.bfloat16)
        # fr layout: [p, k*128 + b] = ft_t[k, 64*core+p, b], bf16, f3 negated
        frc = np.ascontiguousarray(ft_t[:4, sl, :].transpose(1, 0, 2)).copy()
        frc[:, 3, :] *= -1.0
        frc_np = frc.reshape(NS, 4 * B).astype(ml_dtypes.bfloat16)
        in_maps.append({"ft": ft_np, "wt": wt_np, "fr": frc_np})
    return in_maps


# out channel order in the DRAM out tensor columns
OUT_ORDER = [0, 1, 3, 4, 6, 2, 5]


def assemble(results):
    full = np.empty((B, N, 7), np.float32)
    for core, r in enumerate(results):
        o = r["out"].reshape(NS, 7, B)  # (n, slot, b)
        for slot, ch in enumerate(OUT_ORDER):
            full[:, NS * core: NS * (core + 1), ch] = o[:, slot, :].T
    return full


def kernel(combvec, w_dist, w_ener, w_pid, w_extra0, w_extra1, _bench=None):
    in_maps = make_in_maps(combvec, w_dist, w_ener, w_pid, w_extra0, w_extra1)
    nc = _get_nc()
    kw = dict(_bench) if _bench else {}
    res = run_bass_kernel_spmd(nc, in_maps, core_ids=list(range(NCORES)), **kw)
    out = assemble(res.results)
    if _bench is not None:
        kernel.last_results = res
    return out



# revision 7
# speedup vs baseline: 1.0764x; 1.0764x over previous
"""LoLa message-passing kernel for 8 Trainium2 NeuronCores (v2).

Math (identical to the reference):
  ch0 masses      = f3^2 - f2^2 - f1^2 - f0^2
  ch1 ptsq        = f1^2 + f2^2
  ch2 w_ener@f0, ch4 w_pid@f3, ch5 w_extra0@f4, ch6 w_extra1@f5
  ch3 weighted_d  = masses * rowsum(w_dist) + w_dist @ masses
                    + 2*(f0*(w_dist@f0) + f1*(w_dist@f1)
                         + f2*(w_dist@f2) - f3*(w_dist@f3))

Sharding: model-parallel over particles N (64 output rows per core); combvec
replicated (full contraction operand), weights sliced 1/8 per core.

v2 changes vs the 22.3us baseline (all validated against a host-side
precision sim; total rel err ~1.0e-2 vs the 2e-2 gate):
 - Mixed precision: w_ener/w_pid/w_extra0/w_extra1 and f4/f5 ship as
   fp8e4 (matmul allows fp8 stationary x bf16 moving); w_dist and
   f0..f3 stay bf16 (ch3 dominates the output norm). Outputs ship bf16.
   Input bytes 1.18MB -> 852KB per core; output 229KB -> 115KB.
 - fr eliminated: a per-core particle permutation puts this core's own
   64 rows at slot 0 / partitions 0:64, so fr ops read the streamed ft
   in place (-64KB).
 - rowsum(w_dist) is one host-computed bf16 column (drops the
   ones-column trick and its memsets).
 - Transfers are dependency-sorted (w_dist+slot0 first, f4/f5 last:
   their chain is the shallowest) and merged into two DRAM tensors so
   each queue has only 3 input dma_starts (issue cost ~0.7us each).
 - Tail parallelized across ACT/DVE/GPSIMD with per-slot combine trees;
   ch3 ships in its own tiny final DMA.
"""

import sys

if "/opt/trn_rl_repo" not in sys.path:
    sys.path.insert(0, "/opt/trn_rl_repo")

import numpy as np
import ml_dtypes

import concourse.bass as bass
import concourse.mybir as mybir
import concourse.tile as tile
from concourse import bacc
from concourse.bass_utils import run_bass_kernel_spmd

B, N, F = 128, 512, 6
NCORES = 8
NS = N // NCORES  # 64 output rows per core
DT = mybir.dt.float32
BF = mybir.dt.bfloat16
F8 = mybir.dt.float8e4
ALU = mybir.AluOpType
ACTF = mybir.ActivationFunctionType

# bf tensor column layout: [wd stationaries 4*64 | rowsum 1 | f03 4*512]
WD0 = 0          # wd slot s at [s*64, (s+1)*64)
RS0 = 256        # rowsum column (partitions 0:64)
FT0 = 257        # f03 slot s at [FT0 + s*512 + f*128 + b]
BFW = 257 + 4 * 512  # 2305
# f8 tensor column layout: [pairs 4*256 | f45 4*256]
#  pair slot s: [s*256, s*256+128) = [ener|pid], [+128, +256) = [x0|x1]
F45 = 1024       # f45 slot s at [F45 + s*256 + f*128 + b], f in {4,5}
F8W = 2048
# out tensor (128, 640) bf16:
#  partitions 0:64  cols [ch0 | ch1 | ch2 | ch5 | ch3]
#  partitions 64:128 cols [ch4 | ch6] (cols 0:256)
OUTW = 640


def _emit(tc, nc, bf_d, f8_d, out_d):
    with (
        tc.tile_pool(name="sbuf", bufs=1) as sb,
        tc.tile_pool(name="psum", bufs=1, space="PSUM") as ps,
    ):
        # --- persistent SBUF tiles ---
        bf = sb.tile([128, BFW], BF)     # wd slots + rowsum + f03
        f8 = sb.tile([128, F8W], F8)     # fp8 weight pairs + f45
        sq = sb.tile([128, 2048], BF)    # squares of f03, cols s*512+f*128+b
        mt = sb.tile([128, 512], BF)     # masses, cols s*128+b
        ta = sb.tile([128, 512], BF)     # per-slot t_a = sq3-sq2
        tb = sb.tile([128, 512], BF)     # per-slot t_b = sq0+sq1 (gpsimd)
        quad = sb.tile([64, 512], BF)    # fr * psD
        qs = sb.tile([64, 256], BF)      # quad reduction temps
        tmp3 = sb.tile([64, B], DT)      # stt1 out (fp32)
        rs32 = sb.tile([64, 1], DT)      # rowsum converted bf16 -> fp32
        olo = sb.tile([64, 5 * B], BF)   # out staging lo: ch0,ch1,ch2,ch5,ch3
        ohi = sb.tile([128, 2 * B], BF)  # out staging hi (parts 64:128): ch4,ch6

        # --- PSUM tiles (full-bank padded so accumulation groups never
        # share a bank: start=True clears the whole bank's has_written) ---
        psW = ps.tile([128, 512], DT)    # warm-up target
        psD = ps.tile([64, 512], DT)     # w_dist @ [f0..f3], cols f*128+b
        psEP = ps.tile([128, 512], DT)   # [ener|pid] @ [f0 | f3], cols 0:256
        psX = ps.tile([128, 512], DT)    # [x0|x1] @ [f4 | f5], cols 0:256
        psC2 = ps.tile([64, 512], DT)    # w_dist @ masses, cols 0:128

        # --- input DMAs: dependency-sorted; 3 per queue.
        # sync:   wd+rowsum+f03_s0 | f03_s1 | f45(s0,s1)
        # scalar: w8 pairs         | f03_s2 | f03_s3 | f45(s2,s3)
        nc.sync.dma_start(bf[:, 0: FT0 + 512], bf_d[:, 0: FT0 + 512])
        nc.scalar.dma_start(f8[:, 0:1024], f8_d[:, 0:1024])
        nc.sync.dma_start(bf[:, FT0 + 512: FT0 + 1024], bf_d[:, FT0 + 512: FT0 + 1024])
        nc.scalar.dma_start(bf[:, FT0 + 1024: FT0 + 1536], bf_d[:, FT0 + 1024: FT0 + 1536])
        nc.sync.dma_start(f8[:, 1024:1536], f8_d[:, 1024:1536])
        nc.scalar.dma_start(bf[:, FT0 + 1536: BFW], bf_d[:, FT0 + 1536: BFW])
        nc.scalar.dma_start(f8[:, 1536:2048], f8_d[:, 1536:2048])

        def fts(s):  # f03 slot s columns
            return bf[:, FT0 + s * 512: FT0 + (s + 1) * 512]

        def ftf(s, f):  # single feature f (0..3) of slot s
            return bf[:, FT0 + s * 512 + f * 128: FT0 + s * 512 + (f + 1) * 128]

        def f45f(s, f):  # f in {0->f4, 1->f5}
            return f8[:, F45 + s * 256 + f * 128: F45 + s * 256 + (f + 1) * 128]

        def wds(s):
            return bf[:, s * 64: (s + 1) * 64]

        def eps(s):
            return f8[:, s * 256: s * 256 + 128]

        def xps(s):
            return f8[:, s * 256 + 128: s * 256 + 256]

        # --- PE warm-up: dep-free dummy matmuls keep HAM activity from
        # kernel start until the first real operand lands (~11us), so the
        # PE clock ramps 1.2 -> 2.4 GHz before the matmul crunch. ---
        warm = sb.tile([128, 2 * B], BF)
        nc.vector.memset(warm[:], 0.5)
        wmov = warm[:, None, :].to_broadcast([128, 4, 2 * B])
        for i in range(7):
            nc.tensor.matmul(
                psW[:], warm[:, 0:B], wmov[:, :, 0:B], start=i == 0, stop=i == 6
            )

        # --- matmuls, emitted in expected arrival order ---
        def mmDEP(s):
            nc.tensor.matmul(psD[:, :], wds(s), fts(s), start=s == 0, stop=s == 3)
            nc.tensor.matmul(
                psEP[:, 0:128], eps(s), ftf(s, 0), start=s == 0, stop=False
            )
            nc.tensor.matmul(
                psEP[:, 128:256], eps(s), ftf(s, 3), start=False, stop=s == 3
            )

        def mmX(s):
            nc.tensor.matmul(
                psX[:, 0:128], xps(s), f45f(s, 0), start=s == 0, stop=False
            )
            nc.tensor.matmul(
                psX[:, 128:256], xps(s), f45f(s, 1), start=False, stop=s == 3
            )

        def mmC2(s, start=False, stop=False):
            nc.tensor.matmul(
                psC2[:, 0:128], wds(s), mt[:, s * 128: (s + 1) * 128],
                start=start, stop=stop,
            )

        mmDEP(0)
        mmDEP(1)
        mmDEP(2)
        with tc.tile_wait_until(1):
            mmX(0)
            mmX(1)
            mmDEP(3)
        with tc.tile_wait_until(2):
            mmX(2)
            mmX(3)

        # --- masses: sq_f = f^2 per slot; combine tree
        #   t_a = sq3 - sq2 (vector), t_b = sq0 + sq1 (gpsimd),
        #   mt_s = t_a - t_b (vector).
        # Squares: s0,s1 on ACT (lands first), s2,s3 on vector (critical,
        # DVE 2x bf16 is ~2x faster than ACT here). ---
        def sqs(s):
            return sq[:, s * 512: (s + 1) * 512]

        def sqf(s, f):
            return sq[:, s * 512 + f * 128: s * 512 + (f + 1) * 128]

        nc.scalar.activation(sqs(0), fts(0), ACTF.Square)
        nc.scalar.activation(sqs(1), fts(1), ACTF.Square)
        nc.vector.tensor_tensor(out=sqs(2), in0=fts(2), in1=fts(2), op=ALU.mult)
        nc.vector.tensor_tensor(out=sqs(3), in0=fts(3), in1=fts(3), op=ALU.mult)

        for s in range(4):
            nc.gpsimd.tensor_tensor(
                out=tb[:, s * 128: (s + 1) * 128], in0=sqf(s, 0), in1=sqf(s, 1),
                op=ALU.add,
            )
        with tc.tile_wait_until(1):
            for s in range(4):
                nc.vector.tensor_tensor(
                    out=ta[:, s * 128: (s + 1) * 128], in0=sqf(s, 3), in1=sqf(s, 2),
                    op=ALU.subtract,
                )
                nc.vector.tensor_tensor(
                    out=mt[:, s * 128: (s + 1) * 128],
                    in0=ta[:, s * 128: (s + 1) * 128],
                    in1=tb[:, s * 128: (s + 1) * 128],
                    op=ALU.subtract,
                )

        # --- C2 matmuls (read mt, so emitted after the combines above:
        # Tile tracks deps in emission order) ---
        with tc.tile_wait_until(2):
            mmC2(0, start=True)
            mmC2(1)
            mmC2(2)
        with tc.tile_wait_until(3):
            mmC2(3, stop=True)

        # --- own-row channels: ch0 = masses, ch1 = f1^2 + f2^2 ---
        with tc.tile_wait_until(2):
            nc.vector.tensor_copy(rs32[:], bf[0:64, RS0: RS0 + 1])
            nc.vector.tensor_copy(olo[:, 0:B], mt[0:64, 0:B])
            nc.vector.tensor_tensor(
                out=olo[:, B: 2 * B], in0=sq[0:64, 128:256], in1=sq[0:64, 256:384],
                op=ALU.add,
            )

        # --- quad chain: fr (slot0 parts 0:64 of f03) * psD, reduce over f
        # with the f3 term subtracted, then the two stt ops finish ch3. ---
        fr = bf[0:64, FT0: FT0 + 512]
        with tc.tile_wait_until(3):
            nc.vector.tensor_tensor(out=quad[:], in0=fr, in1=psD[:, :], op=ALU.mult)
            nc.vector.tensor_tensor(
                out=qs[:, 0:B], in0=quad[:, 0:B], in1=quad[:, B: 2 * B], op=ALU.add
            )
            nc.vector.tensor_tensor(
                out=qs[:, B: 2 * B], in0=quad[:, 2 * B: 3 * B],
                in1=quad[:, 3 * B: 4 * B], op=ALU.subtract,
            )
            nc.vector.tensor_tensor(
                out=qs[:, 0:B], in0=qs[:, 0:B], in1=qs[:, B: 2 * B], op=ALU.add
            )
        with tc.tile_wait_until(4):
            nc.vector.scalar_tensor_tensor(
                out=tmp3[:],
                in0=mt[0:64, 0:B],
                scalar=rs32[:],
                in1=psC2[:, 0:B],
                op0=ALU.mult,
                op1=ALU.add,
            )
            nc.vector.scalar_tensor_tensor(
                out=olo[:, 4 * B: 5 * B],
                in0=qs[:, 0:B],
                scalar=2.0,
                in1=tmp3[:],
                op0=ALU.mult,
                op1=ALU.add,
            )

        # --- PSUM evacuation on ACT: ch2/ch5 to olo (parts 0:64),
        # ch4/ch6 to ohi (parts 64:128) ---
        with tc.tile_wait_until(2):
            nc.scalar.copy(olo[:, 2 * B: 3 * B], psEP[0:64, 0:B])
            nc.scalar.copy(ohi[64:128, 0:B], psEP[64:128, B: 2 * B])
        with tc.tile_wait_until(3):
            nc.scalar.copy(olo[:, 3 * B: 4 * B], psX[0:64, 0:B])
            nc.scalar.copy(ohi[64:128, B: 2 * B], psX[64:128, B: 2 * B])

        # --- out DMAs: hi on scalar; lo split so ch3 (last ready) ships
        # alone as the final small transfer on sync. ---
        with tc.tile_wait_until(4):
            nc.scalar.dma_start(out_d[64:128, 0: 2 * B], ohi[64:128, :])
            nc.sync.dma_start(out_d[0:64, 0: 4 * B], olo[:, 0: 4 * B])
        with tc.tile_wait_until(5):
            nc.sync.dma_start(out_d[0:64, 4 * B: 5 * B], olo[:, 4 * B: 5 * B])


_NC_CACHE = {}


def _get_nc():
    if "nc" not in _NC_CACHE:
        nc = bacc.Bacc(
            "TRN2", target_bir_lowering=False, debug=False, num_devices=NCORES
        )
        bf_d = nc.dram_tensor("bf", [128, BFW], BF, kind="ExternalInput")
        f8_d = nc.dram_tensor("f8", [128, F8W], F8, kind="ExternalInput")
        out_d = nc.dram_tensor("out", [128, OUTW], BF, kind="ExternalOutput")
        with tile.TileContext(nc) as tc:
            _emit(tc, nc, bf_d.ap(), f8_d.ap(), out_d.ap())
        nc.compile()
        _NC_CACHE["nc"] = nc
    return _NC_CACHE["nc"]


def make_in_maps(combvec, w_dist, w_ener, w_pid, w_extra0, w_extra1):
    ft = np.ascontiguousarray(
        np.transpose(np.asarray(combvec, np.float32), (2, 1, 0))
    )  # (6, N, B) [f, n, b]
    wd = np.asarray(w_dist, np.float32)
    rowsum = wd.sum(axis=1)  # (N,)
    w8list = [
        (0, np.asarray(w_ener, np.float32)),
        (64, np.asarray(w_pid, np.float32)),
        (128, np.asarray(w_extra0, np.float32)),
        (192, np.asarray(w_extra1, np.float32)),
    ]
    in_maps = []
    for core in range(NCORES):
        c0, half = divmod(core, 2)
        own = np.arange(NS * core, NS * (core + 1))
        # slot 0 = own chunk with own rows first; slots 1..3 = other chunks
        ch_rows = np.arange(128 * c0, 128 * (c0 + 1))
        perm0 = np.concatenate([ch_rows[64:], ch_rows[:64]]) if half else ch_rows
        part = [perm0] + [
            np.arange(128 * c, 128 * (c + 1)) for c in range(4) if c != c0
        ]
        part = np.stack(part)  # (4, 128) particle index per (slot, partition)

        bf_np = np.zeros((128, BFW), np.float32)
        wd_own = wd[own]  # (64, N)
        for s in range(4):
            bf_np[:, s * 64: (s + 1) * 64] = wd_own[:, part[s]].T
        bf_np[0:64, RS0] = rowsum[own]
        a = ft[0:4][:, part, :]  # (4f, 4s, 128p, 128b)
        bf_np[:, FT0:BFW] = a.transpose(2, 1, 0, 3).reshape(128, 2048)

        f8_np = np.zeros((128, F8W), np.float32)
        for off, w in w8list:
            wo = w[own]  # (64, N)
            for s in range(4):
                f8_np[:, s * 256 + off: s * 256 + off + 64] = wo[:, part[s]].T
        a45 = ft[4:6][:, part, :]  # (2f, 4s, 128p, 128b)
        f8_np[:, F45:F8W] = a45.transpose(2, 1, 0, 3).reshape(128, 1024)

        in_maps.append(
            {
                "bf": bf_np.astype(ml_dtypes.bfloat16),
                "f8": f8_np.astype(ml_dtypes.float8_e4m3),
            }
        )
    return in_maps


# olo channel order (cols, partitions 0:64), ohi channel order (parts 64:128)
LO_ORDER = [0, 1, 2, 5, 3]
HI_ORDER = [4, 6]


def assemble(results):
    full = np.empty((B, N, 7), np.float32)
    for core, r in enumerate(results):
        o = np.asarray(r["out"]).astype(np.float32)
        lo = o[0:64].reshape(NS, 5, B)
        hi = o[64:128, 0: 2 * B].reshape(NS, 2, B)
        sl = slice(NS * core, NS * (core + 1))
        for i, ch in enumerate(LO_ORDER):
            full[:, sl, ch] = lo[:, i, :].T
        for i, ch in enumerate(HI_ORDER):
            full[:, sl, ch] = hi[:, i, :].T
    return full


def kernel(combvec, w_dist, w_ener, w_pid, w_extra0, w_extra1, _bench=None):
    in_maps = make_in_maps(combvec, w_dist, w_ener, w_pid, w_extra0, w_extra1)
    nc = _get_nc()
    kw = dict(_bench) if _bench else {}
    res = run_bass_kernel_spmd(nc, in_maps, core_ids=list(range(NCORES)), **kw)
    out = assemble(res.results)
    if _bench is not None:
        kernel.last_results = res
    return out
